# revision 4
# baseline (speedup 1.0000x reference)
"""Channel-attention kernel for Trainium2 (8 NeuronCores, data-parallel over batch).

Math: the reference expands x (B,C,T,1) to 8 channels via a 1x1 conv before the
Q@K^T einsum.  Algebraically, with alpha=w1.w2, delta=b1.w2 and
s[b,c]=sum_t x[b,c,t]:

    energy[b,c,e] = alpha*G[b,c,e] + delta*s[b,e] + (terms const along e)
    G[b] = X[b] @ X[b]^T          (X[b] = x[b,:,:,0], shape (C,T))

Terms constant along the e (last) axis cancel in the min-max normalization;
only alpha*G + delta*s_e matters.  This cuts the contraction from T*8 down
to T (the advertised 8x headroom).

v2 design (DMA-roofline oriented; mandatory HBM traffic is 16.4 MB/core
~= 46 us at 358 GB/s, so the kernel is structured to keep the 16 SDMA
engines saturated and everything else overlapped underneath):
  - loads cast f32->bf16 *during* the DMA (SWDGE/gpsimd ring) -- no
    on-chip cast pass, no f32 copy of x in SBUF at all
  - residual is folded into the attention matmul: lhsT = I + gamma*A,
    so PSUM holds the final output chunk (copy + store, no adds)
  - row-sums s via one DVE reduce per pair; rank-1 (delta/alpha)*s_e
    update via a 1-partition matmul into the Gram PSUM group
  - DMA issue spread over all three rings: cast-loads on gpsimd
    (SWDGE), transposes split sync/scalar (the two HWDGE rings),
    stores split sync/scalar
"""

import numpy as np
import ml_dtypes
from contextlib import ExitStack

import concourse.bass as bass
import concourse.tile as tile
from concourse import mybir
from concourse.bass_utils import run_bass_kernel_spmd
from concourse.alu_op_type import AluOpType

F32 = mybir.dt.float32
BF16 = mybir.dt.bfloat16
AX = mybir.AxisListType.X

B, C, T = 64, 64, 4000
NCORES = 8
BPC = B // NCORES          # 8 batches per core
PAIRS = BPC // 2           # 4 pairs of 2 batches
ROWS = BPC * C             # 512 rows of (C,T) per core
TP = 4096                  # T padded to a multiple of 128
NKT = TP // 128            # 32 k-tiles
HALF = TP // 2             # load/transpose half (2048 cols)
NCHUNK = 8
CHW = T // NCHUNK          # 500 (fits one PSUM bank in f32)
QW = T // 4                # 1000-col store quarters
EPS = 1e-8


def _body(ctx, tc, out_ap, x_ap, idf_ap, alpha, doa, gamma):
    nc = tc.nc

    singles = ctx.enter_context(tc.tile_pool(name="singles", bufs=1))
    xbp = ctx.enter_context(tc.tile_pool(name="xbp", bufs=3))
    xtp = ctx.enter_context(tc.tile_pool(name="xtp", bufs=3))
    obp = ctx.enter_context(tc.tile_pool(name="obp", bufs=2))
    attp = ctx.enter_context(tc.tile_pool(name="attp", bufs=2))
    smalls = ctx.enter_context(tc.tile_pool(name="smalls", bufs=3))

    ps_s = ctx.enter_context(tc.tile_pool(name="ps_s", bufs=2, space="PSUM"))
    ps_g = ctx.enter_context(tc.tile_pool(name="ps_g", bufs=2, space="PSUM"))
    ps_o = ctx.enter_context(tc.tile_pool(name="ps_o", bufs=4, space="PSUM"))

    ident_f32 = singles.tile([128, 128], F32)
    nc.sync.dma_start(ident_f32[:], idf_ap)
    ident_bf = singles.tile([128, 128], BF16)
    nc.vector.tensor_copy(ident_bf[:], ident_f32[:])
    ones_row = singles.tile([1, 128], BF16)
    nc.vector.memset(ones_row[:], 1.0)
    # preload the ACT function tables during the ramp
    warm_act = singles.tile([1, 2], F32)
    nc.scalar.activation(
        warm_act[:], ones_row[0:1, 0:2], mybir.ActivationFunctionType.Exp
    )

    st = [{} for _ in range(PAIRS)]

    def stL(p):
        """cast-loads (SWDGE ring; pure DMA, no on-chip cast pass)."""
        v = st[p]
        rows = slice(p * 128, (p + 1) * 128)
        x_bf = xbp.tile([128, TP], BF16)
        nc.gpsimd.dma_start(x_bf[:, 0:HALF], x_ap[rows, 0:HALF])
        nc.vector.memset(x_bf[:, T:TP], 0.0)
        nc.gpsimd.dma_start(x_bf[:, HALF:T], x_ap[rows, HALF:T])
        v["x_bf"] = x_bf

    def stT(p):
        """xbar transposes (one per HWDGE ring) + the row-sum reduce."""
        v = st[p]
        x_bf = v["x_bf"]
        xt = xtp.tile([128, TP], BF16)
        nc.sync.dma_start_transpose(
            xt[:, 0:HALF].rearrange("q (k f) -> q k f", f=128),
            x_bf[:, 0:HALF],
        )
        nc.scalar.dma_start_transpose(
            xt[:, HALF:TP].rearrange("q (k f) -> q k f", f=128),
            x_bf[:, HALF:TP],
        )
        s_col = smalls.tile([128, 1], F32, tag="scol")
        nc.vector.tensor_reduce(s_col[:], x_bf[:, 0:T], axis=AX, op=AluOpType.add)
        v["xt"] = xt
        v["s_col"] = s_col

    def stGx(p):
        """Gram matmuls + s-row prep + the aux rank-1 update (PE-dense).
        The s-transpose sits between the halves so the PE never waits on
        work that isn't already due (half 1's transpose lands ~when the
        DVE reduce does)."""
        v = st[p]
        xt = v["xt"]
        psum_g = ps_g.tile([128, 128], F32, tag="g")
        for kt in range(NKT // 2):
            base = kt * 128
            nc.tensor.matmul(
                psum_g[:],
                lhsT=xt[:, base: base + 128],
                rhs=xt[:, base: base + 128],
                start=(kt == 0),
                stop=False,
            )
        st_ps = ps_s.tile([1, 128], F32, tag="st")
        nc.tensor.transpose(st_ps[:], v["s_col"][:], ident_f32[:])
        rhs_aux = smalls.tile([1, 128], BF16, tag="aux")
        nc.vector.tensor_scalar_mul(rhs_aux[:], st_ps[:], doa)
        for kt in range(NKT // 2, NKT):
            base = kt * 128
            nc.tensor.matmul(
                psum_g[:],
                lhsT=xt[:, base: base + 128],
                rhs=xt[:, base: base + 128],
                start=False,
                stop=False,
            )
        nc.tensor.matmul(
            psum_g[:],
            lhsT=ones_row[:],
            rhs=rhs_aux[:],
            start=False,
            stop=True,
            skip_group_check=True,
        )
        v["psum_g"] = psum_g

    def stGy(p):
        """energy extraction + min-max softmax -> attention lhsT with the
        residual identity folded in (M = I + gamma*A, block-diagonal)."""
        v = st[p]
        psum_g = v["psum_g"]
        # Diagonal (64,64) blocks, scaled by alpha -> energy (128, 64)
        e_sb = smalls.tile([128, 64], F32, tag="esb")
        nc.vector.tensor_scalar_mul(e_sb[0:64, :], psum_g[0:64, 0:64], alpha)
        nc.vector.tensor_scalar_mul(
            e_sb[64:128, :], psum_g[64:128, 64:128], alpha
        )

        # min-max normalize along free axis, then softmax (normalized values
        # live in [0,1], so no max-subtraction is needed before exp)
        rmax = smalls.tile([128, 1], F32, tag="rmax")
        nc.vector.tensor_reduce(rmax[:], e_sb[:], axis=AX, op=AluOpType.max)
        rmin = smalls.tile([128, 1], F32, tag="rmin")
        nc.vector.tensor_reduce(rmin[:], e_sb[:], axis=AX, op=AluOpType.min)
        den = smalls.tile([128, 1], F32, tag="den")
        nc.vector.tensor_scalar(
            den[:], rmax[:], scalar1=rmin[:], scalar2=EPS,
            op0=AluOpType.subtract, op1=AluOpType.add,
        )
        rden = smalls.tile([128, 1], F32, tag="rden")
        nc.vector.reciprocal(rden[:], den[:])
        nbias = smalls.tile([128, 1], F32, tag="nbias")
        nc.vector.scalar_tensor_tensor(
            nbias[:], in0=rmin[:], scalar=-1.0, in1=rden[:],
            op0=AluOpType.mult, op1=AluOpType.mult,
        )
        ex = smalls.tile([128, 64], F32, tag="ex")
        nc.scalar.activation(
            ex[:], e_sb[:], mybir.ActivationFunctionType.Exp,
            bias=nbias[:], scale=rden[:],
        )
        ssum = smalls.tile([128, 1], F32, tag="ssum")
        nc.vector.tensor_reduce(ssum[:], ex[:], axis=AX, op=AluOpType.add)
        rsum = smalls.tile([128, 1], F32, tag="rsum")
        nc.vector.reciprocal(rsum[:], ssum[:])

        latt0 = attp.tile([128, 128], BF16, tag="latt0")
        nc.vector.memset(latt0[:], 0.0)
        nc.vector.tensor_scalar(
            latt0[0:64, 0:64], ex[0:64, :], scalar1=rsum[0:64], scalar2=gamma,
            op0=AluOpType.mult, op1=AluOpType.mult,
        )
        nc.vector.tensor_scalar(
            latt0[64:128, 64:128], ex[64:128, :], scalar1=rsum[64:128],
            scalar2=gamma, op0=AluOpType.mult, op1=AluOpType.mult,
        )
        latt = attp.tile([128, 128], BF16, tag="latt")
        nc.vector.tensor_add(latt[:], latt0[:], ident_bf[:])
        v["latt"] = latt

    def stA(p):
        """output chunks: PSUM holds the final result (residual folded into
        the matmul); alternate DVE/ACT copies, stores split sync/scalar."""
        v = st[p]
        rows = slice(p * 128, (p + 1) * 128)
        x_bf, latt = v["x_bf"], v["latt"]
        out_sb = obp.tile([128, T], F32)
        for ch in range(NCHUNK):
            cols = slice(ch * CHW, (ch + 1) * CHW)
            psum_o = ps_o.tile([128, CHW], F32, tag="o")
            nc.tensor.matmul(
                psum_o[:], lhsT=latt[:], rhs=x_bf[:, cols], start=True,
                stop=True,
            )
            if ch % 2 == 0:
                nc.vector.tensor_copy(out_sb[:, cols], psum_o[:])
            else:
                nc.scalar.copy(out_sb[:, cols], psum_o[:])
                q = ch // 2
                qcols = slice(q * QW, (q + 1) * QW)
                ring = nc.sync if q % 2 == 0 else nc.scalar
                ring.dma_start(out_ap[rows, qcols], out_sb[:, qcols])
        v.clear()

    # software-pipelined schedule, hand-skewed so the PE instruction stream
    # never waits on same-pair DVE/ACT work and the SDMA engines stay fed
    sched = [
        (stL, 0), (stL, 1), (stT, 0), (stL, 2),
        (stGx, 0), (stT, 1),
        (stGx, 1), (stGy, 0),
        (stA, 0), (stL, 3), (stT, 2),
        (stGx, 2), (stGy, 1),
        (stA, 1), (stT, 3),
        (stGx, 3), (stGy, 2),
        (stA, 2),
        (stGy, 3),
        (stA, 3),
    ]
    for fn, p in sched:
        fn(p)


_MODULE_CACHE = {}


def _build_module(alpha, doa, gamma):
    key = (alpha, doa, gamma)
    if key in _MODULE_CACHE:
        return _MODULE_CACHE[key]
    nc = bass.Bass(
        "TRN2", target_bir_lowering=False, debug=False, num_devices=NCORES
    )
    x_ap = nc.dram_tensor("x", (ROWS, T), F32, kind="ExternalInput").ap()
    idf_ap = nc.dram_tensor("idf", (128, 128), F32, kind="ExternalInput").ap()
    out_ap = nc.dram_tensor("out", (ROWS, T), F32, kind="ExternalOutput").ap()
    with tile.TileContext(nc) as tc, ExitStack() as ctx:
        _body(ctx, tc, out_ap, x_ap, idf_ap, alpha, doa, gamma)
    if _LEGALIZE_WAITS:
        _split_waits(nc)
    _MODULE_CACHE[key] = nc
    return nc


# The wait-split legalization confuses CoreSim's bookkeeping (hand-built
# NoOps bypass nc.inst_map); tests flip this off for simulation runs.
_LEGALIZE_WAITS = True


def _split_waits(nc):
    """walrus TRN2 codegen allows only ONE sync wait per instruction; when
    Tile emits more (e.g. PSUM slot reuse: previous-writer completion +
    previous-reader), hoist the extras onto same-engine NoOps inserted
    immediately before — the sequencer dispatches in order, so the blocking
    semantics are identical."""
    nid = [0]
    for f in nc.m.functions:
        for block in f.blocks:
            out = []
            for inst in block.instructions:
                si = getattr(inst, "sync_info", None)
                if (
                    si is not None
                    and si.on_wait
                    and len(si.on_wait) > 1
                    and type(inst).__name__ != "InstNoOp"
                ):
                    waits = list(si.on_wait)
                    for w in waits[:-1]:
                        nid[0] += 1
                        out.append(
                            mybir.InstNoOp(
                                name=f"{inst.name}-wsplit{nid[0]}",
                                engine=inst.engine,
                                ins=[],
                                outs=[],
                                sync_info=mybir.SyncInfo(
                                    on_wait=[w], on_update=[]
                                ),
                                text_hint="wait-split",
                                bass_nofuse=True,
                            )
                        )
                    inst.sync_info = mybir.SyncInfo(
                        on_wait=waits[-1:], on_update=list(si.on_update)
                    )
                out.append(inst)
            block.instructions[:] = out


def _prepare(inputs):
    x = np.ascontiguousarray(
        np.asarray(inputs["x"], dtype=np.float32).reshape(B * C, T)
    )
    w1 = np.asarray(inputs["w1"], dtype=np.float64)
    b1 = np.asarray(inputs["b1"], dtype=np.float64)
    w2 = np.asarray(inputs["w2"], dtype=np.float64)
    b2 = np.asarray(inputs["b2"], dtype=np.float64)
    gamma = float(np.asarray(inputs["gamma"]))
    alpha = float(w1 @ w2)
    delta = float(b1 @ w2)
    assert abs(alpha) > 1e-12, "degenerate alpha not supported"
    nc = _build_module(alpha, delta / alpha, gamma)
    ident_f = np.eye(128, dtype=np.float32)
    in_maps = [
        {"x": x[i * ROWS:(i + 1) * ROWS], "idf": ident_f}
        for i in range(NCORES)
    ]
    return nc, in_maps


def kernel(**inputs):
    nc, in_maps = _prepare(inputs)
    res = run_bass_kernel_spmd(nc, in_maps, core_ids=list(range(NCORES)))
    out = np.concatenate([res.results[i]["out"] for i in range(NCORES)], axis=0)
    return out.reshape(B, C, T, 1)


# revision 8
# speedup vs baseline: 1.1137x; 1.1137x over previous
"""Channel-attention kernel for Trainium2 (8 NeuronCores, data-parallel over batch).

Math: the reference expands x (B,C,T,1) to 8 channels via a 1x1 conv before the
Q@K^T einsum.  Algebraically, with alpha=w1.w2, delta=b1.w2 and
s[b,c]=sum_t x[b,c,t]:

    energy[b,c,e] = alpha*G[b,c,e] + delta*s[b,e] + (terms const along e)
    G[b] = X[b] @ X[b]^T          (X[b] = x[b,:,:,0], shape (C,T))

Terms constant along the e (last) axis cancel in the min-max normalization;
only alpha*G + delta*s_e matters.  This cuts the contraction from T*8 down
to T (the advertised 8x headroom).

v3 design (DMA-roofline oriented; mandatory HBM traffic is 16.4 MB/core
~= 46 us at 358 GB/s, so the kernel is structured to keep the 16 SDMA
engines saturated and everything else overlapped underneath):
  - f32 loads split across the two HWDGE rings (sync/scalar); x_f32
    lives only until the cast (bufs=3 lets loads run ~3 pairs ahead,
    keeping HBM reads continuous)
  - cast f32->bf16 on DVE (half 0) and ACT (half 1), each with
    accum_out producing its half of the row-sums s -- no reduce pass
    (SWDGE cast-during-DMA was tried and measured ~96 GB/s: 3.5x too
    slow, it throttles the whole pipeline)
  - residual is folded into the attention matmul: lhsT = I + gamma*A,
    so PSUM holds the final output chunk (copy + store, no adds and no
    second life for x_f32)
  - rank-1 (delta/alpha)*s_e update via a 1-partition matmul appended
    to the Gram PSUM accumulation group
  - transposes and stores also split across both HWDGE rings
"""

import numpy as np
import ml_dtypes
from contextlib import ExitStack

import concourse.bass as bass
import concourse.tile as tile
from concourse import mybir
from concourse.bass_utils import run_bass_kernel_spmd
from concourse.alu_op_type import AluOpType

F32 = mybir.dt.float32
BF16 = mybir.dt.bfloat16
AX = mybir.AxisListType.X

B, C, T = 64, 64, 4000
NCORES = 8
BPC = B // NCORES          # 8 batches per core
PAIRS = BPC // 2           # 4 pairs of 2 batches
ROWS = BPC * C             # 512 rows of (C,T) per core
TP = 4096                  # T padded to a multiple of 128
NKT = TP // 128            # 32 k-tiles
HALF = TP // 2             # load/transpose half (2048 cols)
NCHUNK = 8
CHW = T // NCHUNK          # 500 (fits one PSUM bank in f32)
QW = T // 4                # 1000-col store quarters
EPS = 1e-8


def _body(ctx, tc, out_ap, x_ap, idf_ap, alpha, doa, gamma):
    nc = tc.nc

    singles = ctx.enter_context(tc.tile_pool(name="singles", bufs=1))
    xfp = ctx.enter_context(tc.tile_pool(name="xfp", bufs=3))
    xbp = ctx.enter_context(tc.tile_pool(name="xbp", bufs=3))
    xtp = ctx.enter_context(tc.tile_pool(name="xtp", bufs=3))
    obp = ctx.enter_context(tc.tile_pool(name="obp", bufs=2))
    attp = ctx.enter_context(tc.tile_pool(name="attp", bufs=2))
    smalls = ctx.enter_context(tc.tile_pool(name="smalls", bufs=3))

    ps_s = ctx.enter_context(tc.tile_pool(name="ps_s", bufs=2, space="PSUM"))
    ps_g = ctx.enter_context(tc.tile_pool(name="ps_g", bufs=2, space="PSUM"))
    ps_o = ctx.enter_context(tc.tile_pool(name="ps_o", bufs=4, space="PSUM"))

    ident_f32 = singles.tile([128, 128], F32)
    nc.sync.dma_start(ident_f32[:], idf_ap)
    ident_bf = singles.tile([128, 128], BF16)
    nc.vector.tensor_copy(ident_bf[:], ident_f32[:])
    ones_row = singles.tile([1, 128], BF16)
    nc.vector.memset(ones_row[:], 1.0)
    # preload the ACT function tables during the ramp
    warm_act = singles.tile([1, 2], F32)
    nc.scalar.activation(
        warm_act[:], ones_row[0:1, 0:2], mybir.ActivationFunctionType.Exp
    )

    st = [{} for _ in range(PAIRS)]

    def stL(p):
        """f32 loads, one half per HWDGE ring; pad memset on gpsimd."""
        v = st[p]
        rows = slice(p * 128, (p + 1) * 128)
        x_f32 = xfp.tile([128, T], F32)
        x_bf = xbp.tile([128, TP], BF16)
        nc.sync.dma_start(x_f32[:, 0:HALF], x_ap[rows, 0:HALF])
        nc.scalar.dma_start(x_f32[:, HALF:T], x_ap[rows, HALF:T])
        nc.gpsimd.memset(x_bf[:, T:TP], 0.0)
        v["x_f32"] = x_f32
        v["x_bf"] = x_bf

    def stC(p):
        """casts with row-sum accumulation: DVE half 0, ACT half 1."""
        v = st[p]
        x_f32, x_bf = v["x_f32"], v["x_bf"]
        s_ab = smalls.tile([128, 2], F32, tag="sab")
        nc.vector.tensor_scalar(
            x_bf[:, 0:HALF], x_f32[:, 0:HALF], scalar1=1.0, scalar2=0.0,
            op0=AluOpType.mult, op1=AluOpType.add, accum_out=s_ab[:, 0:1],
        )
        nc.scalar.activation(
            x_bf[:, HALF:T], x_f32[:, HALF:T],
            mybir.ActivationFunctionType.Copy, accum_out=s_ab[:, 1:2],
        )
        s_col = smalls.tile([128, 1], F32, tag="scol")
        nc.vector.tensor_reduce(s_col[:], s_ab[:], axis=AX, op=AluOpType.add)
        v["s_col"] = s_col

    def stT(p):
        """xbar transposes, one per HWDGE ring."""
        v = st[p]
        x_bf = v["x_bf"]
        xt = xtp.tile([128, TP], BF16)
        nc.sync.dma_start_transpose(
            xt[:, 0:HALF].rearrange("q (k f) -> q k f", f=128),
            x_bf[:, 0:HALF],
        )
        nc.scalar.dma_start_transpose(
            xt[:, HALF:TP].rearrange("q (k f) -> q k f", f=128),
            x_bf[:, HALF:TP],
        )
        v["xt"] = xt

    def stGx(p):
        """Gram matmuls + s-row prep + the aux rank-1 update (PE-dense).
        The s-transpose sits between the halves so the PE never waits on
        work that isn't already due (half 1's transpose lands ~when the
        DVE reduce does)."""
        v = st[p]
        xt = v["xt"]
        psum_g = ps_g.tile([128, 128], F32, tag="g")
        for kt in range(NKT // 2):
            base = kt * 128
            nc.tensor.matmul(
                psum_g[:],
                lhsT=xt[:, base: base + 128],
                rhs=xt[:, base: base + 128],
                start=(kt == 0),
                stop=False,
            )
        st_ps = ps_s.tile([1, 128], F32, tag="st")
        nc.tensor.transpose(st_ps[:], v["s_col"][:], ident_f32[:])
        rhs_aux = smalls.tile([1, 128], BF16, tag="aux")
        nc.vector.tensor_scalar_mul(rhs_aux[:], st_ps[:], doa)
        for kt in range(NKT // 2, NKT):
            base = kt * 128
            nc.tensor.matmul(
                psum_g[:],
                lhsT=xt[:, base: base + 128],
                rhs=xt[:, base: base + 128],
                start=False,
                stop=False,
            )
        nc.tensor.matmul(
            psum_g[:],
            lhsT=ones_row[:],
            rhs=rhs_aux[:],
            start=False,
            stop=True,
            skip_group_check=True,
        )
        v["psum_g"] = psum_g

    def stGy(p):
        """energy extraction + min-max softmax -> attention lhsT with the
        residual identity folded in (M = I + gamma*A, block-diagonal)."""
        v = st[p]
        psum_g = v["psum_g"]
        # Diagonal (64,64) blocks, scaled by alpha -> energy (128, 64)
        e_sb = smalls.tile([128, 64], F32, tag="esb")
        nc.vector.tensor_scalar_mul(e_sb[0:64, :], psum_g[0:64, 0:64], alpha)
        nc.vector.tensor_scalar_mul(
            e_sb[64:128, :], psum_g[64:128, 64:128], alpha
        )

        # min-max normalize along free axis, then softmax (normalized values
        # live in [0,1], so no max-subtraction is needed before exp)
        rmax = smalls.tile([128, 1], F32, tag="rmax")
        nc.vector.tensor_reduce(rmax[:], e_sb[:], axis=AX, op=AluOpType.max)
        rmin = smalls.tile([128, 1], F32, tag="rmin")
        nc.vector.tensor_reduce(rmin[:], e_sb[:], axis=AX, op=AluOpType.min)
        den = smalls.tile([128, 1], F32, tag="den")
        nc.vector.tensor_scalar(
            den[:], rmax[:], scalar1=rmin[:], scalar2=EPS,
            op0=AluOpType.subtract, op1=AluOpType.add,
        )
        rden = smalls.tile([128, 1], F32, tag="rden")
        nc.vector.reciprocal(rden[:], den[:])
        nbias = smalls.tile([128, 1], F32, tag="nbias")
        nc.vector.scalar_tensor_tensor(
            nbias[:], in0=rmin[:], scalar=-1.0, in1=rden[:],
            op0=AluOpType.mult, op1=AluOpType.mult,
        )
        ex = smalls.tile([128, 64], F32, tag="ex")
        nc.scalar.activation(
            ex[:], e_sb[:], mybir.ActivationFunctionType.Exp,
            bias=nbias[:], scale=rden[:],
        )
        ssum = smalls.tile([128, 1], F32, tag="ssum")
        nc.vector.tensor_reduce(ssum[:], ex[:], axis=AX, op=AluOpType.add)
        rsum = smalls.tile([128, 1], F32, tag="rsum")
        nc.vector.reciprocal(rsum[:], ssum[:])

        latt0 = attp.tile([128, 128], BF16, tag="latt0")
        nc.vector.memset(latt0[:], 0.0)
        nc.vector.tensor_scalar(
            latt0[0:64, 0:64], ex[0:64, :], scalar1=rsum[0:64], scalar2=gamma,
            op0=AluOpType.mult, op1=AluOpType.mult,
        )
        nc.vector.tensor_scalar(
            latt0[64:128, 64:128], ex[64:128, :], scalar1=rsum[64:128],
            scalar2=gamma, op0=AluOpType.mult, op1=AluOpType.mult,
        )
        latt = attp.tile([128, 128], BF16, tag="latt")
        nc.vector.tensor_add(latt[:], latt0[:], ident_bf[:])
        v["latt"] = latt

    def stA(p):
        """output chunks: PSUM holds the final result (residual folded into
        the matmul); alternate DVE/ACT copies, stores split sync/scalar."""
        v = st[p]
        rows = slice(p * 128, (p + 1) * 128)
        x_bf, latt = v["x_bf"], v["latt"]
        out_sb = obp.tile([128, T], F32)
        for ch in range(NCHUNK):
            cols = slice(ch * CHW, (ch + 1) * CHW)
            psum_o = ps_o.tile([128, CHW], F32, tag="o")
            nc.tensor.matmul(
                psum_o[:], lhsT=latt[:], rhs=x_bf[:, cols], start=True,
                stop=True,
            )
            if ch % 2 == 0:
                nc.vector.tensor_copy(out_sb[:, cols], psum_o[:])
            else:
                nc.scalar.copy(out_sb[:, cols], psum_o[:])
                q = ch // 2
                qcols = slice(q * QW, (q + 1) * QW)
                ring = nc.sync if q % 2 == 0 else nc.scalar
                ring.dma_start(out_ap[rows, qcols], out_sb[:, qcols])
        v.clear()

    # software-pipelined schedule, hand-skewed so the PE instruction stream
    # never waits on same-pair DVE/ACT work and the SDMA engines stay fed
    sched = [
        (stL, 0), (stL, 1), (stC, 0), (stT, 0), (stL, 2),
        (stGx, 0), (stC, 1), (stT, 1),
        (stGx, 1), (stGy, 0),
        (stA, 0), (stL, 3), (stC, 2), (stT, 2),
        (stGx, 2), (stGy, 1),
        (stA, 1), (stC, 3), (stT, 3),
        (stGx, 3), (stGy, 2),
        (stA, 2),
        (stGy, 3),
        (stA, 3),
    ]
    for fn, p in sched:
        fn(p)


_MODULE_CACHE = {}


def _build_module(alpha, doa, gamma):
    key = (alpha, doa, gamma)
    if key in _MODULE_CACHE:
        return _MODULE_CACHE[key]
    nc = bass.Bass(
        "TRN2", target_bir_lowering=False, debug=False, num_devices=NCORES
    )
    x_ap = nc.dram_tensor("x", (ROWS, T), F32, kind="ExternalInput").ap()
    idf_ap = nc.dram_tensor("idf", (128, 128), F32, kind="ExternalInput").ap()
    out_ap = nc.dram_tensor("out", (ROWS, T), F32, kind="ExternalOutput").ap()
    with tile.TileContext(nc) as tc, ExitStack() as ctx:
        _body(ctx, tc, out_ap, x_ap, idf_ap, alpha, doa, gamma)
    if _LEGALIZE_WAITS:
        _split_waits(nc)
    _MODULE_CACHE[key] = nc
    return nc


# The wait-split legalization confuses CoreSim's bookkeeping (hand-built
# NoOps bypass nc.inst_map); tests flip this off for simulation runs.
_LEGALIZE_WAITS = True


def _split_waits(nc):
    """walrus TRN2 codegen allows only ONE sync wait per instruction; when
    Tile emits more (e.g. PSUM slot reuse: previous-writer completion +
    previous-reader), hoist the extras onto same-engine NoOps inserted
    immediately before — the sequencer dispatches in order, so the blocking
    semantics are identical."""
    nid = [0]
    for f in nc.m.functions:
        for block in f.blocks:
            out = []
            for inst in block.instructions:
                si = getattr(inst, "sync_info", None)
                if (
                    si is not None
                    and si.on_wait
                    and len(si.on_wait) > 1
                    and type(inst).__name__ != "InstNoOp"
                ):
                    waits = list(si.on_wait)
                    for w in waits[:-1]:
                        nid[0] += 1
                        out.append(
                            mybir.InstNoOp(
                                name=f"{inst.name}-wsplit{nid[0]}",
                                engine=inst.engine,
                                ins=[],
                                outs=[],
                                sync_info=mybir.SyncInfo(
                                    on_wait=[w], on_update=[]
                                ),
                                text_hint="wait-split",
                                bass_nofuse=True,
                            )
                        )
                    inst.sync_info = mybir.SyncInfo(
                        on_wait=waits[-1:], on_update=list(si.on_update)
                    )
                out.append(inst)
            block.instructions[:] = out


def _prepare(inputs):
    x = np.ascontiguousarray(
        np.asarray(inputs["x"], dtype=np.float32).reshape(B * C, T)
    )
    w1 = np.asarray(inputs["w1"], dtype=np.float64)
    b1 = np.asarray(inputs["b1"], dtype=np.float64)
    w2 = np.asarray(inputs["w2"], dtype=np.float64)
    b2 = np.asarray(inputs["b2"], dtype=np.float64)
    gamma = float(np.asarray(inputs["gamma"]))
    alpha = float(w1 @ w2)
    delta = float(b1 @ w2)
    assert abs(alpha) > 1e-12, "degenerate alpha not supported"
    nc = _build_module(alpha, delta / alpha, gamma)
    ident_f = np.eye(128, dtype=np.float32)
    in_maps = [
        {"x": x[i * ROWS:(i + 1) * ROWS], "idf": ident_f}
        for i in range(NCORES)
    ]
    return nc, in_maps


def kernel(**inputs):
    nc, in_maps = _prepare(inputs)
    res = run_bass_kernel_spmd(nc, in_maps, core_ids=list(range(NCORES)))
    out = np.concatenate([res.results[i]["out"] for i in range(NCORES)], axis=0)
    return out.reshape(B, C, T, 1)
